# revision 6
# baseline (speedup 1.0000x reference)
"""Contrastive loss kernel for Trainium2 (8 NeuronCores, Bass/Tile).

Strategy (data-parallel over rows of embeddings1, compute-roofline focused):
  - Host prep: L2-normalize both embedding sets (f64), compute the exact
    diagonal logits there (O(N*D), ~0.05% of total FLOPs), quantize the
    normalized embeddings to fp8-e4m3 and pre-transpose so the device does
    nothing but the N^2 GEMM + exp + sums.
  - Core c owns rows [c*512, (c+1)*512) of e1 ("i").  Its tile of the logit
    matrix is [i=512, j=4096], computed 128 i's at a time with e1^T blocks as
    the PE stationary operand and e2^T streaming (fp8 DoubleRow: contraction
    256 per matmul -> 2x PE throughput, 216 ns / [128x512] tile measured).
  - j-group-outer loop (8 groups x 1024 j): each group's 4 i-tiles are
    matmul'd + exp'd, then that group's column-sum work (DVE adds + a
    ones-matmul partition reduction) runs while later groups compute.
  - ACT Exp evacuates each 2-bank PSUM group ([128 i, 1024 j]) into bf16
    SBUF with scale=10 (1/temperature); its accum_out gives the per-i row
    partial sums for free (i is on partitions).
  - Startup hiding: input DMAs are issued from the vector/scalar/gpsimd
    queues (the sync queue spends ~7 us on framework preamble), and dummy
    warm-up matmuls keep the PE busy so the HAM clock-gate reaches full
    rate before the real GEMM starts.
  - Host combines: rows = sum of the 8 group partials; colsum = sum over
    cores; denominators exclude the diagonal via the exact host ldiag.

Per-core outputs: rows [128, 16] (p, it*4+g), colp [1, 4096].
"""

import os
import sys

import numpy as np

for _p in ("/root/.axon_site", "/root/.axon_site/_ro/trn_rl_repo",
           "/root/.axon_site/_ro/pypackages", "/opt/trn_rl_repo"):
    if os.path.isdir(_p) and _p not in sys.path:
        sys.path.append(_p)

import ml_dtypes

N, D = 4096, 1024
NCORES = 8
CH = N // NCORES          # 512 rows of e1 per core
IT = CH // 128            # 4 i-tiles
JC = N // 512             # 8 j-chunks
KK = D // 256             # 4 contraction super-tiles (pairs of 128)
NG = N // 1024            # 4 j-groups of 1024 (2 chunks each)
INV_T = 10.0              # 1 / temperature
NWARM = 8                 # PE warm-up matmuls

_CACHE = {}


def _legalize_waits(nc, cap=1):
    """Split >cap semaphore waits per instruction onto preceding NOPs.

    The walrus build in this container rejects instructions carrying more
    than ~2 sync waits ("Too many sync wait commands"); Tile emits up to
    12 on the final barrier drain.  Hoisting the excess waits onto NOPs
    issued just before, on the same engine queue, is semantics-preserving
    (the engine is in-order, so waiting earlier is safe).
    """
    import concourse.mybir as mybir
    nid = 0
    for f in nc.m.functions:
        for b in f.blocks:
            insts = b.instructions
            i = 0
            while i < len(insts):
                inst = insts[i]
                si = inst.sync_info
                if si is not None and si.on_wait and len(si.on_wait) > cap:
                    waits = list(si.on_wait)
                    inst.sync_info = mybir.SyncInfo(
                        on_wait=waits[-cap:], on_update=list(si.on_update))
                    excess = waits[:-cap]
                    pos = i
                    for j in range(0, len(excess), cap):
                        nop = mybir.InstNoOp(
                            name=f"I-waitnop-{nid}", ins=[], outs=[])
                        nid += 1
                        nop.engine = inst.engine
                        nop.sync_info = mybir.SyncInfo(
                            on_wait=excess[j:j + cap], on_update=[])
                        insts.insert(pos, nop)
                        pos += 1
                        i += 1
                i += 1
    return nc


def build_nc(legalize=True):
    import concourse.bass as bass
    import concourse.mybir as mybir
    import concourse.tile as tile
    from contextlib import ExitStack

    fp32 = mybir.dt.float32
    bf16 = mybir.dt.bfloat16
    f8 = mybir.dt.float8e4
    AF = mybir.ActivationFunctionType
    DR = mybir.MatmulPerfMode.DoubleRow
    ts = bass.ts

    nc = bass.Bass(trn_type="TRN2")
    # e1t: normalized e1^T for this core, [p, kk, pr, i] with d = kk*256+pr*128+p
    e1t_d = nc.dram_tensor("e1t", [128, KK * 2 * CH], f8, kind="ExternalInput")
    # e2t: full normalized e2^T, [p, jc, kk, pr, jl] with j = jc*512+jl
    e2t_d = nc.dram_tensor("e2t", [128, JC * KK * 2 * 512], f8,
                           kind="ExternalInput")
    rows_d = nc.dram_tensor("rows", [128, IT * NG], fp32,
                            kind="ExternalOutput")
    colp_d = nc.dram_tensor("colp", [1, N], fp32, kind="ExternalOutput")

    with ExitStack() as ctx:
        tc = ctx.enter_context(tile.TileContext(nc))
        res = ctx.enter_context(tc.tile_pool(name="res", bufs=1))
        pml = ctx.enter_context(tc.tile_pool(name="pml", bufs=3, space="PSUM"))
        pcol = ctx.enter_context(tc.tile_pool(name="pcol", bufs=1,
                                              space="PSUM"))

        e1t_sb = res.tile([128, KK, 2, CH], f8)
        e2t_sb = res.tile([128, JC, KK, 2, 512], f8)
        exps_sb = res.tile([128, IT, N], bf16)
        colacc = res.tile([128, N], bf16)
        rows_sb = res.tile([128, IT * NG], fp32)
        colp_sb = res.tile([1, N], fp32)
        ones_bf = res.tile([128, 1], bf16)
        scratch = res.tile([128, 512], bf16)

        # memsets first so the PE warm-up can start immediately
        nc.vector.memset(ones_bf, 1.0)
        nc.vector.memset(scratch, 1.0)

        # ---- input DMAs, spread across otherwise-idle engine queues ----
        # (the sync queue spends ~7us on framework preamble; only gpsimd /
        # SP / Activation may initiate DMAs -> use scalar + gpsimd, with
        # the first-needed chunks issued first on each)
        nc.scalar.dma_start(out=e1t_sb, in_=e1t_d[:, :])
        qs = [nc.scalar, nc.gpsimd] * (JC // 2)
        for jc in range(JC):
            qs[jc].dma_start(out=e2t_sb[:, jc],
                             in_=e2t_d[:, ts(jc, KK * 2 * 512)])

        # ---- PE warm-up: HAM clock-gate needs ~3.4us of busy to unthrottle
        warm = pcol.tile([128, 2, 512], fp32, tag="cps")
        for w in range(NWARM):
            nc.tensor.matmul(warm[0:1, 0, :], lhsT=ones_bf, rhs=scratch,
                             start=True, stop=True)

        # ---- main loop: j-group outer, i-tile inner ----
        # pending colsum ones-matmuls, emitted 2 groups late so the PE's
        # in-order queue never waits on the DVE adds feeding them
        pending = []

        def emit_colsum(g):
            for j2 in range(2):
                c = 2 * g + j2
                sl = ts(c, 512)
                cps = pcol.tile([128, 2, 512], fp32, tag="cps")
                nc.tensor.matmul(cps[0:1, j2, :], lhsT=ones_bf,
                                 rhs=colacc[:, sl], start=True, stop=True)
                nc.vector.tensor_copy(out=colp_sb[:, sl], in_=cps[0:1, j2, :])

        for g in range(NG):
            for it in range(IT):
                pg = pml.tile([128, 2, 512], fp32, tag="pg")
                for kk in range(KK):
                    for j2 in range(2):
                        jc = 2 * g + j2
                        nc.tensor.matmul(
                            pg[:, j2, :],
                            lhsT=e1t_sb[:, kk, :, ts(it, 128)],
                            rhs=e2t_sb[:, jc, kk, :, :],
                            start=(kk == 0), stop=(kk == KK - 1),
                            perf_mode=DR)
                nc.scalar.activation(
                    out=exps_sb[:, it, ts(g, 1024)], in_=pg,
                    func=AF.Exp, scale=INV_T,
                    accum_out=rows_sb[:, it * NG + g:it * NG + g + 1])
                # column-sum accumulation for this group (DVE, overlapped)
                if it >= 1:
                    for j2 in range(2):
                        sl = ts(2 * g + j2, 512)
                        if it == 1:
                            nc.vector.tensor_add(out=colacc[:, sl],
                                                 in0=exps_sb[:, 0, sl],
                                                 in1=exps_sb[:, 1, sl])
                        else:
                            nc.vector.tensor_add(out=colacc[:, sl],
                                                 in0=colacc[:, sl],
                                                 in1=exps_sb[:, it, sl])
            pending.append(g)
            if len(pending) > 1:
                emit_colsum(pending.pop(0))
        nc.sync.dma_start(out=rows_d[:, :], in_=rows_sb)
        for g in pending:
            emit_colsum(g)
        nc.sync.dma_start(out=colp_d[:, :], in_=colp_sb)
    return _legalize_waits(nc) if legalize else nc


def _get_nc():
    if "nc" not in _CACHE:
        _CACHE["nc"] = build_nc()
    return _CACHE["nc"]


def _run(in_maps, trace=False, **kw):
    from concourse.bass_utils import run_bass_kernel_spmd
    return run_bass_kernel_spmd(_get_nc(), in_maps,
                                core_ids=list(range(NCORES)),
                                trace=trace, **kw)


def kernel(embeddings1, embeddings2, _trace=False, _full_result=False):
    e1 = np.asarray(embeddings1, dtype=np.float32)
    e2 = np.asarray(embeddings2, dtype=np.float32)
    assert e1.shape == (N, D) and e2.shape == (N, D)
    f8 = ml_dtypes.float8_e4m3

    e1n = e1.astype(np.float64)
    e1n /= np.maximum(np.linalg.norm(e1n, axis=1, keepdims=True), 1e-12)
    e2n = e2.astype(np.float64)
    e2n /= np.maximum(np.linalg.norm(e2n, axis=1, keepdims=True), 1e-12)
    ldiag = INV_T * np.sum(e1n * e2n, axis=1)

    q1 = e1n.astype(np.float32).astype(f8)
    q2 = e2n.astype(np.float32).astype(f8)
    # e2t[p, jc, kk, pr, jl] = q2.T[kk*256+pr*128+p, jc*512+jl]
    e2t = np.ascontiguousarray(
        q2.T.reshape(KK, 2, 128, JC, 512)
            .transpose(2, 3, 0, 1, 4).reshape(128, -1))

    in_maps = []
    for c in range(NCORES):
        q1c = q1[c * CH:(c + 1) * CH]
        # e1t[p, kk, pr, i] = q1c.T[kk*256+pr*128+p, i]
        e1t = np.ascontiguousarray(
            q1c.T.reshape(KK, 2, 128, CH).transpose(2, 0, 1, 3)
               .reshape(128, -1))
        in_maps.append({"e1t": e1t, "e2t": e2t})

    bres = _run(in_maps, trace=_trace)
    outs = bres.results

    rows = np.concatenate([
        np.asarray(o["rows"], dtype=np.float64)
          .reshape(128, IT, NG).sum(axis=2).T.reshape(-1)
        for o in outs])
    colsum = np.zeros(N, dtype=np.float64)
    for o in outs:
        colsum += np.asarray(o["colp"], dtype=np.float64).reshape(-1)

    ed = np.exp(ldiag)
    row_denom = rows - ed
    col_denom = colsum - ed
    sim12 = float(np.sum(ldiag - np.log(row_denom)))
    sim21 = float(np.sum(ldiag - np.log(col_denom)))
    result = (np.float32(-sim12), np.float32(-sim21))
    if _full_result:
        return result, bres
    return result


# revision 9
# speedup vs baseline: 1.0783x; 1.0783x over previous
"""Contrastive loss kernel for Trainium2 (8 NeuronCores, Bass/Tile).

Strategy (data-parallel over rows of embeddings1, compute-roofline focused):
  - Host prep: L2-normalize both embedding sets (f64), compute the exact
    diagonal logits there (O(N*D), ~0.05% of total FLOPs), quantize the
    normalized embeddings to fp8-e4m3 and pre-transpose so the device does
    nothing but the N^2 GEMM + exp + sums.
  - Core c owns rows [c*512, (c+1)*512) of e1 ("i").  Its tile of the logit
    matrix is [i=512, j=4096], computed 128 i's at a time with e1^T blocks as
    the PE stationary operand and e2^T streaming (fp8 DoubleRow: contraction
    256 per matmul -> 2x PE throughput, 216 ns / [128x512] tile measured).
  - j-group-outer loop (8 groups x 1024 j): each group's 4 i-tiles are
    matmul'd + exp'd, then that group's column-sum work (DVE adds + a
    ones-matmul partition reduction) runs while later groups compute.
  - ACT Exp evacuates each 2-bank PSUM group ([128 i, 1024 j]) into bf16
    SBUF with scale=10 (1/temperature); its accum_out gives the per-i row
    partial sums for free (i is on partitions).
  - Startup hiding: input DMAs are issued from the vector/scalar/gpsimd
    queues (the sync queue spends ~7 us on framework preamble), and dummy
    warm-up matmuls keep the PE busy so the HAM clock-gate reaches full
    rate before the real GEMM starts.
  - Host combines: rows = sum of the 8 group partials; colsum = sum over
    cores; denominators exclude the diagonal via the exact host ldiag.

Per-core outputs: rows [128, 16] (p, it*4+g), colp [1, 4096].
"""

import os
import sys

import numpy as np

for _p in ("/root/.axon_site", "/root/.axon_site/_ro/trn_rl_repo",
           "/root/.axon_site/_ro/pypackages", "/opt/trn_rl_repo"):
    if os.path.isdir(_p) and _p not in sys.path:
        sys.path.append(_p)

import ml_dtypes

N, D = 4096, 1024
NCORES = 8
CH = N // NCORES          # 512 rows of e1 per core
IT = CH // 128            # 4 i-tiles
JC = N // 512             # 8 j-chunks
KK = D // 256             # 4 contraction super-tiles (pairs of 128)
NG = N // 1024            # 4 j-groups of 1024 (2 chunks each)
INV_T = 10.0              # 1 / temperature
NWARM = 10                # PE warm-up matmuls

_CACHE = {}


def _legalize_waits(nc, cap=1):
    """Split >cap semaphore waits per instruction onto preceding NOPs.

    The walrus build in this container rejects instructions carrying more
    than ~2 sync waits ("Too many sync wait commands"); Tile emits up to
    12 on the final barrier drain.  Hoisting the excess waits onto NOPs
    issued just before, on the same engine queue, is semantics-preserving
    (the engine is in-order, so waiting earlier is safe).
    """
    import concourse.mybir as mybir
    nid = 0
    for f in nc.m.functions:
        for b in f.blocks:
            insts = b.instructions
            i = 0
            while i < len(insts):
                inst = insts[i]
                si = inst.sync_info
                if si is not None and si.on_wait and len(si.on_wait) > cap:
                    waits = list(si.on_wait)
                    inst.sync_info = mybir.SyncInfo(
                        on_wait=waits[-cap:], on_update=list(si.on_update))
                    excess = waits[:-cap]
                    pos = i
                    for j in range(0, len(excess), cap):
                        nop = mybir.InstNoOp(
                            name=f"I-waitnop-{nid}", ins=[], outs=[])
                        nid += 1
                        nop.engine = inst.engine
                        nop.sync_info = mybir.SyncInfo(
                            on_wait=excess[j:j + cap], on_update=[])
                        insts.insert(pos, nop)
                        pos += 1
                        i += 1
                i += 1
    return nc


def build_nc(legalize=True):
    import concourse.bass as bass
    import concourse.mybir as mybir
    import concourse.tile as tile
    from contextlib import ExitStack

    fp32 = mybir.dt.float32
    bf16 = mybir.dt.bfloat16
    f8 = mybir.dt.float8e4
    AF = mybir.ActivationFunctionType
    DR = mybir.MatmulPerfMode.DoubleRow
    ts = bass.ts

    nc = bass.Bass(trn_type="TRN2")
    # e1t: normalized e1^T for this core, [p, kk, pr, i] with d = kk*256+pr*128+p
    e1t_d = nc.dram_tensor("e1t", [128, KK * 2 * CH], f8, kind="ExternalInput")
    # e2t: full normalized e2^T, [p, jc, kk, pr, jl] with j = jc*512+jl
    e2t_d = nc.dram_tensor("e2t", [128, JC * KK * 2 * 512], f8,
                           kind="ExternalInput")
    rows_d = nc.dram_tensor("rows", [128, IT * NG], fp32,
                            kind="ExternalOutput")
    colp_d = nc.dram_tensor("colp", [1, N], fp32, kind="ExternalOutput")

    with ExitStack() as ctx:
        tc = ctx.enter_context(tile.TileContext(nc))
        res = ctx.enter_context(tc.tile_pool(name="res", bufs=1))
        pml = ctx.enter_context(tc.tile_pool(name="pml", bufs=3, space="PSUM"))
        pcol = ctx.enter_context(tc.tile_pool(name="pcol", bufs=2,
                                              space="PSUM"))

        e1t_sb = res.tile([128, KK, 2, CH], f8)
        e2t_sb = res.tile([128, JC, KK, 2, 512], f8)
        exps_sb = res.tile([128, IT, N], bf16)
        colacc = res.tile([128, N], bf16)
        rows_sb = res.tile([128, IT * NG], fp32)
        colp_sb = res.tile([1, N], fp32)
        ones_bf = res.tile([128, 1], bf16)
        scratch = res.tile([128, 512], bf16)

        # memsets first so the PE warm-up can start immediately
        nc.vector.memset(ones_bf, 1.0)
        nc.vector.memset(scratch, 1.0)

        # ---- input DMAs: one ordered stream on the scalar queue ----
        # A single queue keeps chunk completion in issue order at full HBM
        # bandwidth (split queues fair-share and delay the first chunk).
        # gpsimd is avoided entirely: touching it makes its 12 exit DRAINs
        # cost ~1.2us each (DGE drain) instead of ~0.2us.
        nc.scalar.dma_start(out=e1t_sb, in_=e1t_d[:, :])
        for jc in range(JC):
            nc.scalar.dma_start(out=e2t_sb[:, jc],
                                in_=e2t_d[:, ts(jc, KK * 2 * 512)])

        # ---- PE warm-up: HAM clock-gate needs ~3.4us of busy to unthrottle,
        # and these also bridge the PE until the first input chunks land
        warm = pcol.tile([128, 512], fp32, tag="cps")
        for w in range(NWARM):
            nc.tensor.matmul(warm[0:1, :], lhsT=ones_bf, rhs=scratch,
                             start=True, stop=True)

        # ---- main loop: j-group outer, i-tile inner ----
        # colsum ones-matmuls are emitted 1 group late (so the PE queue
        # never waits on the DVE adds feeding them) and the PSUM->SBUF
        # copies one slot later still (so the DVE queue never blocks on a
        # not-yet-run ones-matmul while later adds wait behind it).  Copy
        # emission precedes the matmul that reuses the rotating pcol
        # buffer, keeping Tile's WAR tracking sound.
        cps_tiles = {}

        def emit_colsum_mm(g):
            for j2 in range(2):
                c = 2 * g + j2
                cps = pcol.tile([128, 512], fp32, tag="cps")
                cps_tiles[c] = cps
                nc.tensor.matmul(cps[0:1, :], lhsT=ones_bf,
                                 rhs=colacc[:, ts(c, 512)],
                                 start=True, stop=True)

        def emit_colsum_copy(g):
            for j2 in range(2):
                c = 2 * g + j2
                nc.vector.tensor_copy(out=colp_sb[:, ts(c, 512)],
                                      in_=cps_tiles.pop(c)[0:1, :])

        for g in range(NG):
            for it in range(IT):
                pg = pml.tile([128, 2, 512], fp32, tag="pg")
                for kk in range(KK):
                    for j2 in range(2):
                        jc = 2 * g + j2
                        nc.tensor.matmul(
                            pg[:, j2, :],
                            lhsT=e1t_sb[:, kk, :, ts(it, 128)],
                            rhs=e2t_sb[:, jc, kk, :, :],
                            start=(kk == 0), stop=(kk == KK - 1),
                            perf_mode=DR)
                nc.scalar.activation(
                    out=exps_sb[:, it, ts(g, 1024)], in_=pg,
                    func=AF.Exp, scale=INV_T,
                    accum_out=rows_sb[:, it * NG + g:it * NG + g + 1])
                # column-sum accumulation for this group (DVE, overlapped)
                if it >= 1:
                    for j2 in range(2):
                        sl = ts(2 * g + j2, 512)
                        if it == 1:
                            nc.vector.tensor_add(out=colacc[:, sl],
                                                 in0=exps_sb[:, 0, sl],
                                                 in1=exps_sb[:, 1, sl])
                        else:
                            nc.vector.tensor_add(out=colacc[:, sl],
                                                 in0=colacc[:, sl],
                                                 in1=exps_sb[:, it, sl])
            if g >= 2:
                emit_colsum_copy(g - 2)
            if g >= 1:
                emit_colsum_mm(g - 1)
        nc.sync.dma_start(out=rows_d[:, :], in_=rows_sb)
        emit_colsum_copy(NG - 2)
        emit_colsum_mm(NG - 1)
        emit_colsum_copy(NG - 1)
        nc.sync.dma_start(out=colp_d[:, :], in_=colp_sb)
    return _legalize_waits(nc) if legalize else nc


def _get_nc():
    if "nc" not in _CACHE:
        _CACHE["nc"] = build_nc()
    return _CACHE["nc"]


def _run(in_maps, trace=False, **kw):
    from concourse.bass_utils import run_bass_kernel_spmd
    return run_bass_kernel_spmd(_get_nc(), in_maps,
                                core_ids=list(range(NCORES)),
                                trace=trace, **kw)


def kernel(embeddings1, embeddings2, _trace=False, _full_result=False):
    e1 = np.asarray(embeddings1, dtype=np.float32)
    e2 = np.asarray(embeddings2, dtype=np.float32)
    assert e1.shape == (N, D) and e2.shape == (N, D)
    f8 = ml_dtypes.float8_e4m3

    e1n = e1.astype(np.float64)
    e1n /= np.maximum(np.linalg.norm(e1n, axis=1, keepdims=True), 1e-12)
    e2n = e2.astype(np.float64)
    e2n /= np.maximum(np.linalg.norm(e2n, axis=1, keepdims=True), 1e-12)
    ldiag = INV_T * np.sum(e1n * e2n, axis=1)

    q1 = e1n.astype(np.float32).astype(f8)
    q2 = e2n.astype(np.float32).astype(f8)
    # e2t[p, jc, kk, pr, jl] = q2.T[kk*256+pr*128+p, jc*512+jl]
    e2t = np.ascontiguousarray(
        q2.T.reshape(KK, 2, 128, JC, 512)
            .transpose(2, 3, 0, 1, 4).reshape(128, -1))

    in_maps = []
    for c in range(NCORES):
        q1c = q1[c * CH:(c + 1) * CH]
        # e1t[p, kk, pr, i] = q1c.T[kk*256+pr*128+p, i]
        e1t = np.ascontiguousarray(
            q1c.T.reshape(KK, 2, 128, CH).transpose(2, 0, 1, 3)
               .reshape(128, -1))
        in_maps.append({"e1t": e1t, "e2t": e2t})

    bres = _run(in_maps, trace=_trace)
    outs = bres.results

    rows = np.concatenate([
        np.asarray(o["rows"], dtype=np.float64)
          .reshape(128, IT, NG).sum(axis=2).T.reshape(-1)
        for o in outs])
    colsum = np.zeros(N, dtype=np.float64)
    for o in outs:
        colsum += np.asarray(o["colp"], dtype=np.float64).reshape(-1)

    ed = np.exp(ldiag)
    row_denom = rows - ed
    col_denom = colsum - ed
    sim12 = float(np.sum(ldiag - np.log(row_denom)))
    sim21 = float(np.sum(ldiag - np.log(col_denom)))
    result = (np.float32(-sim12), np.float32(-sim21))
    if _full_result:
        return result, bres
    return result


# revision 10
# speedup vs baseline: 1.1075x; 1.0271x over previous
"""Contrastive loss kernel for Trainium2 (8 NeuronCores, Bass/Tile).

Strategy (data-parallel over rows of embeddings1, compute-roofline focused):
  - Host prep: L2-normalize both embedding sets (f64), compute the exact
    diagonal logits there (O(N*D), ~0.05% of total FLOPs), quantize the
    normalized embeddings to fp8-e4m3 and pre-transpose so the device does
    nothing but the N^2 GEMM + exp + sums.
  - Core c owns rows [c*512, (c+1)*512) of e1 ("i").  Its tile of the logit
    matrix is [i=512, j=4096], computed 128 i's at a time with e1^T blocks as
    the PE stationary operand and e2^T streaming (fp8 DoubleRow: contraction
    256 per matmul -> 2x PE throughput, 216 ns / [128x512] tile measured).
  - j-group-outer loop (8 groups x 1024 j): each group's 4 i-tiles are
    matmul'd + exp'd, then that group's column-sum work (DVE adds + a
    ones-matmul partition reduction) runs while later groups compute.
  - ACT Exp evacuates each 2-bank PSUM group ([128 i, 1024 j]) into bf16
    SBUF with scale=10 (1/temperature); its accum_out gives the per-i row
    partial sums for free (i is on partitions).
  - Startup hiding: input DMAs are issued from the vector/scalar/gpsimd
    queues (the sync queue spends ~7 us on framework preamble), and dummy
    warm-up matmuls keep the PE busy so the HAM clock-gate reaches full
    rate before the real GEMM starts.
  - Host combines: rows = sum of the 8 group partials; colsum = sum over
    cores; denominators exclude the diagonal via the exact host ldiag.

Per-core outputs: rows [128, 16] (p, it*4+g), colp [1, 4096].
"""

import os
import sys

import numpy as np

for _p in ("/root/.axon_site", "/root/.axon_site/_ro/trn_rl_repo",
           "/root/.axon_site/_ro/pypackages", "/opt/trn_rl_repo"):
    if os.path.isdir(_p) and _p not in sys.path:
        sys.path.append(_p)

import ml_dtypes

N, D = 4096, 1024
NCORES = 8
CH = N // NCORES          # 512 rows of e1 per core
IT = CH // 128            # 4 i-tiles
JC = N // 512             # 8 j-chunks
KK = D // 256             # 4 contraction super-tiles (pairs of 128)
NG = N // 1024            # 4 j-groups of 1024 (2 chunks each)
INV_T = 10.0              # 1 / temperature
NWARM = 30                # PE warm-up matmuls (N=128, ~107ns cold)

_CACHE = {}


def _legalize_waits(nc, cap=1):
    """Split >cap semaphore waits per instruction onto preceding NOPs.

    The walrus build in this container rejects instructions carrying more
    than ~2 sync waits ("Too many sync wait commands"); Tile emits up to
    12 on the final barrier drain.  Hoisting the excess waits onto NOPs
    issued just before, on the same engine queue, is semantics-preserving
    (the engine is in-order, so waiting earlier is safe).
    """
    import concourse.mybir as mybir
    nid = 0
    for f in nc.m.functions:
        for b in f.blocks:
            insts = b.instructions
            i = 0
            while i < len(insts):
                inst = insts[i]
                si = inst.sync_info
                if si is not None and si.on_wait and len(si.on_wait) > cap:
                    waits = list(si.on_wait)
                    inst.sync_info = mybir.SyncInfo(
                        on_wait=waits[-cap:], on_update=list(si.on_update))
                    excess = waits[:-cap]
                    pos = i
                    for j in range(0, len(excess), cap):
                        nop = mybir.InstNoOp(
                            name=f"I-waitnop-{nid}", ins=[], outs=[])
                        nid += 1
                        nop.engine = inst.engine
                        nop.sync_info = mybir.SyncInfo(
                            on_wait=excess[j:j + cap], on_update=[])
                        insts.insert(pos, nop)
                        pos += 1
                        i += 1
                i += 1
    return nc


def build_nc(legalize=True):
    import concourse.bass as bass
    import concourse.mybir as mybir
    import concourse.tile as tile
    from contextlib import ExitStack

    fp32 = mybir.dt.float32
    bf16 = mybir.dt.bfloat16
    f8 = mybir.dt.float8e4
    AF = mybir.ActivationFunctionType
    DR = mybir.MatmulPerfMode.DoubleRow
    ts = bass.ts

    nc = bass.Bass(trn_type="TRN2")
    # e1t: normalized e1^T for this core, [p, kk, pr, i] with d = kk*256+pr*128+p
    e1t_d = nc.dram_tensor("e1t", [128, KK * 2 * CH], f8, kind="ExternalInput")
    # e2t: full normalized e2^T, [p, jc, kk, pr, jl] with j = jc*512+jl
    e2t_d = nc.dram_tensor("e2t", [128, JC * KK * 2 * 512], f8,
                           kind="ExternalInput")
    NGR = 5                   # startup groups [0],[1] then pairs
    rows_d = nc.dram_tensor("rows", [128, IT * NGR], fp32,
                            kind="ExternalOutput")
    colp_d = nc.dram_tensor("colp", [1, N], fp32, kind="ExternalOutput")

    with ExitStack() as ctx:
        tc = ctx.enter_context(tile.TileContext(nc))
        res = ctx.enter_context(tc.tile_pool(name="res", bufs=1))
        pml = ctx.enter_context(tc.tile_pool(name="pml", bufs=3, space="PSUM"))
        pcol = ctx.enter_context(tc.tile_pool(name="pcol", bufs=2,
                                              space="PSUM"))

        e1t_sb = res.tile([128, KK, 2, CH], f8)
        e2t_sb = res.tile([128, JC, KK, 2, 512], f8)
        exps_sb = res.tile([128, IT, N], bf16)
        colacc = res.tile([128, N], bf16)
        rows_sb = res.tile([128, IT * NGR], fp32)
        colp_sb = res.tile([1, N], fp32)
        ones_bf = res.tile([128, 1], bf16)
        scratch = res.tile([128, 512], bf16)

        # memsets first so the PE warm-up can start immediately
        nc.vector.memset(ones_bf, 1.0)
        nc.vector.memset(scratch, 1.0)

        # ---- input DMAs: one ordered stream on the sync queue ----
        # A single queue keeps chunk completion in issue order at full HBM
        # bandwidth (split queues fair-share and delay the first chunk),
        # and sync is otherwise idle so the scalar queue stays free for
        # the ACT stream.  gpsimd is avoided entirely: touching it makes
        # its 12 exit DRAINs cost ~1.2us each (DGE drain) instead of ~0.2us.
        nc.sync.dma_start(out=e1t_sb, in_=e1t_d[:, :])
        for jc in range(JC):
            nc.sync.dma_start(out=e2t_sb[:, jc],
                              in_=e2t_d[:, ts(jc, KK * 2 * 512)])

        # ---- PE warm-up: HAM clock-gate needs ~3.4us of busy to unthrottle,
        # and these also bridge the PE until the first input chunks land.
        # Short N=128 matmuls (~107ns cold) keep the bridge fine-grained so
        # the first real matmul starts almost immediately once data lands.
        warm = pcol.tile([128, 512], fp32, tag="cps")
        for w in range(NWARM):
            nc.tensor.matmul(warm[0:1, 0:128], lhsT=ones_bf,
                             rhs=scratch[:, 0:128], start=True, stop=True)

        # ---- main loop: j-group outer, i-tile inner ----
        # colsum ones-matmuls are emitted 1 group late (so the PE queue
        # never waits on the DVE adds feeding them) and the PSUM->SBUF
        # copies one slot later still (so the DVE queue never blocks on a
        # not-yet-run ones-matmul while later adds wait behind it).  Copy
        # emission precedes the matmul that reuses the rotating pcol
        # buffer, keeping Tile's WAR tracking sound.
        GROUPS = [[0], [1], [2, 3], [4, 5], [6, 7]]
        NGR = len(GROUPS)
        cps_tiles = {}

        def emit_colsum_mm(gi):
            for c in GROUPS[gi]:
                cps = pcol.tile([128, 512], fp32, tag="cps")
                cps_tiles[c] = cps
                nc.tensor.matmul(cps[0:1, :], lhsT=ones_bf,
                                 rhs=colacc[:, ts(c, 512)],
                                 start=True, stop=True)

        def emit_colsum_copy(gi):
            for c in GROUPS[gi]:
                nc.vector.tensor_copy(out=colp_sb[:, ts(c, 512)],
                                      in_=cps_tiles.pop(c)[0:1, :])
                # piecewise output DMA: only the last chunk's 2KB transfer
                # remains on the critical tail
                nc.sync.dma_start(out=colp_d[:, ts(c, 512)],
                                  in_=colp_sb[:, ts(c, 512)])

        for gi, chunks in enumerate(GROUPS):
            nch = len(chunks)
            for it in range(IT):
                pg = pml.tile([128, 2, 512], fp32, tag="pg")
                for kk in range(KK):
                    for j2, jc in enumerate(chunks):
                        nc.tensor.matmul(
                            pg[:, j2, :],
                            lhsT=e1t_sb[:, kk, :, ts(it, 128)],
                            rhs=e2t_sb[:, jc, kk, :, :],
                            start=(kk == 0), stop=(kk == KK - 1),
                            perf_mode=DR)
                slot = it * NGR + gi
                if nch == 2:
                    # wide ACT amortizes the ~300ns fixed cost; accum_out
                    # gives the row partial sums in fp32 for free
                    nc.scalar.activation(
                        out=exps_sb[:, it, chunks[0] * 512:
                                    (chunks[-1] + 1) * 512],
                        in_=pg, func=AF.Exp, scale=INV_T,
                        accum_out=rows_sb[:, slot:slot + 1])
                else:
                    # narrow startup group: skip the accumulator read
                    # (ACT headroom is tight) and row-reduce on the DVE
                    nc.scalar.activation(
                        out=exps_sb[:, it, ts(chunks[0], 512)],
                        in_=pg[:, 0, :], func=AF.Exp, scale=INV_T)
                    nc.vector.reduce_sum(
                        out=rows_sb[:, slot:slot + 1],
                        in_=exps_sb[:, it, ts(chunks[0], 512)],
                        axis=mybir.AxisListType.X)
                # column-sum accumulation for this group (DVE, overlapped)
                if it >= 1:
                    for jc in chunks:
                        sl = ts(jc, 512)
                        if it == 1:
                            nc.vector.tensor_add(out=colacc[:, sl],
                                                 in0=exps_sb[:, 0, sl],
                                                 in1=exps_sb[:, 1, sl])
                        else:
                            nc.vector.tensor_add(out=colacc[:, sl],
                                                 in0=colacc[:, sl],
                                                 in1=exps_sb[:, it, sl])
            if gi >= 2:
                emit_colsum_copy(gi - 2)
            if gi >= 1:
                emit_colsum_mm(gi - 1)
        nc.sync.dma_start(out=rows_d[:, :], in_=rows_sb)
        emit_colsum_copy(NGR - 2)
        emit_colsum_mm(NGR - 1)
        emit_colsum_copy(NGR - 1)
    return _legalize_waits(nc) if legalize else nc


def _get_nc():
    if "nc" not in _CACHE:
        _CACHE["nc"] = build_nc()
    return _CACHE["nc"]


def _run(in_maps, trace=False, **kw):
    from concourse.bass_utils import run_bass_kernel_spmd
    return run_bass_kernel_spmd(_get_nc(), in_maps,
                                core_ids=list(range(NCORES)),
                                trace=trace, **kw)


def kernel(embeddings1, embeddings2, _trace=False, _full_result=False):
    e1 = np.asarray(embeddings1, dtype=np.float32)
    e2 = np.asarray(embeddings2, dtype=np.float32)
    assert e1.shape == (N, D) and e2.shape == (N, D)
    f8 = ml_dtypes.float8_e4m3

    e1n = e1.astype(np.float64)
    e1n /= np.maximum(np.linalg.norm(e1n, axis=1, keepdims=True), 1e-12)
    e2n = e2.astype(np.float64)
    e2n /= np.maximum(np.linalg.norm(e2n, axis=1, keepdims=True), 1e-12)
    ldiag = INV_T * np.sum(e1n * e2n, axis=1)

    q1 = e1n.astype(np.float32).astype(f8)
    q2 = e2n.astype(np.float32).astype(f8)
    # e2t[p, jc, kk, pr, jl] = q2.T[kk*256+pr*128+p, jc*512+jl]
    e2t = np.ascontiguousarray(
        q2.T.reshape(KK, 2, 128, JC, 512)
            .transpose(2, 3, 0, 1, 4).reshape(128, -1))

    in_maps = []
    for c in range(NCORES):
        q1c = q1[c * CH:(c + 1) * CH]
        # e1t[p, kk, pr, i] = q1c.T[kk*256+pr*128+p, i]
        e1t = np.ascontiguousarray(
            q1c.T.reshape(KK, 2, 128, CH).transpose(2, 0, 1, 3)
               .reshape(128, -1))
        in_maps.append({"e1t": e1t, "e2t": e2t})

    bres = _run(in_maps, trace=_trace)
    outs = bres.results

    rows = np.concatenate([
        np.asarray(o["rows"], dtype=np.float64)
          .reshape(128, IT, 5).sum(axis=2).T.reshape(-1)
        for o in outs])
    colsum = np.zeros(N, dtype=np.float64)
    for o in outs:
        colsum += np.asarray(o["colp"], dtype=np.float64).reshape(-1)

    ed = np.exp(ldiag)
    row_denom = rows - ed
    col_denom = colsum - ed
    sim12 = float(np.sum(ldiag - np.log(row_denom)))
    sim21 = float(np.sum(ldiag - np.log(col_denom)))
    result = (np.float32(-sim12), np.float32(-sim21))
    if _full_result:
        return result, bres
    return result


# revision 11
# speedup vs baseline: 1.1126x; 1.0046x over previous
"""Contrastive loss kernel for Trainium2 (8 NeuronCores, Bass/Tile).

Strategy (data-parallel over rows of embeddings1, compute-roofline focused):
  - Host prep: L2-normalize both embedding sets (f64), compute the exact
    diagonal logits there (O(N*D), ~0.05% of total FLOPs), quantize the
    normalized embeddings to fp8-e4m3 and pre-transpose so the device does
    nothing but the N^2 GEMM + exp + sums.
  - Core c owns rows [c*512, (c+1)*512) of e1 ("i").  Its tile of the logit
    matrix is [i=512, j=4096], computed 128 i's at a time with e1^T blocks as
    the PE stationary operand and e2^T streaming (fp8 DoubleRow: contraction
    256 per matmul -> 2x PE throughput, 216 ns / [128x512] tile measured).
  - j-group-outer loop (8 groups x 1024 j): each group's 4 i-tiles are
    matmul'd + exp'd, then that group's column-sum work (DVE adds + a
    ones-matmul partition reduction) runs while later groups compute.
  - ACT Exp evacuates each 2-bank PSUM group ([128 i, 1024 j]) into bf16
    SBUF with scale=10 (1/temperature); its accum_out gives the per-i row
    partial sums for free (i is on partitions).
  - Startup hiding: input DMAs are issued from the vector/scalar/gpsimd
    queues (the sync queue spends ~7 us on framework preamble), and dummy
    warm-up matmuls keep the PE busy so the HAM clock-gate reaches full
    rate before the real GEMM starts.
  - Host combines: rows = sum of the 8 group partials; colsum = sum over
    cores; denominators exclude the diagonal via the exact host ldiag.

Per-core outputs: rows [128, 16] (p, it*4+g), colp [1, 4096].
"""

import os
import sys

import numpy as np

for _p in ("/root/.axon_site", "/root/.axon_site/_ro/trn_rl_repo",
           "/root/.axon_site/_ro/pypackages", "/opt/trn_rl_repo"):
    if os.path.isdir(_p) and _p not in sys.path:
        sys.path.append(_p)

import ml_dtypes

N, D = 4096, 1024
NCORES = 8
CH = N // NCORES          # 512 rows of e1 per core
IT = CH // 128            # 4 i-tiles
JC = N // 512             # 8 j-chunks
KK = D // 256             # 4 contraction super-tiles (pairs of 128)
NG = N // 1024            # 4 j-groups of 1024 (2 chunks each)
INV_T = 10.0              # 1 / temperature
NWARM = 24                # PE warm-up matmuls (N=128, ~107ns cold)

_CACHE = {}


def _legalize_waits(nc, cap=1):
    """Split >cap semaphore waits per instruction onto preceding NOPs.

    The walrus build in this container rejects instructions carrying more
    than ~2 sync waits ("Too many sync wait commands"); Tile emits up to
    12 on the final barrier drain.  Hoisting the excess waits onto NOPs
    issued just before, on the same engine queue, is semantics-preserving
    (the engine is in-order, so waiting earlier is safe).
    """
    import concourse.mybir as mybir
    nid = 0
    for f in nc.m.functions:
        for b in f.blocks:
            insts = b.instructions
            i = 0
            while i < len(insts):
                inst = insts[i]
                si = inst.sync_info
                if si is not None and si.on_wait and len(si.on_wait) > cap:
                    waits = list(si.on_wait)
                    inst.sync_info = mybir.SyncInfo(
                        on_wait=waits[-cap:], on_update=list(si.on_update))
                    excess = waits[:-cap]
                    pos = i
                    for j in range(0, len(excess), cap):
                        nop = mybir.InstNoOp(
                            name=f"I-waitnop-{nid}", ins=[], outs=[])
                        nid += 1
                        nop.engine = inst.engine
                        nop.sync_info = mybir.SyncInfo(
                            on_wait=excess[j:j + cap], on_update=[])
                        insts.insert(pos, nop)
                        pos += 1
                        i += 1
                i += 1
    return nc


def build_nc(legalize=True):
    import concourse.bass as bass
    import concourse.mybir as mybir
    import concourse.tile as tile
    from contextlib import ExitStack

    fp32 = mybir.dt.float32
    bf16 = mybir.dt.bfloat16
    f8 = mybir.dt.float8e4
    AF = mybir.ActivationFunctionType
    DR = mybir.MatmulPerfMode.DoubleRow
    ts = bass.ts

    nc = bass.Bass(trn_type="TRN2")
    # e1t: normalized e1^T for this core, it-major so the first i-tile's
    # slice can land before the rest: [p, it, kk, pr, il], d = kk*256+pr*128+p
    e1t_d = nc.dram_tensor("e1t", [128, KK * 2 * CH], f8, kind="ExternalInput")
    # e2t: full normalized e2^T, [p, jc, kk, pr, jl] with j = jc*512+jl
    e2t_d = nc.dram_tensor("e2t", [128, JC * KK * 2 * 512], f8,
                           kind="ExternalInput")
    NGR = 6                   # single-chunk groups at both ends
    rows_d = nc.dram_tensor("rows", [128, IT * NGR], fp32,
                            kind="ExternalOutput")
    colp_d = nc.dram_tensor("colp", [1, N], fp32, kind="ExternalOutput")

    with ExitStack() as ctx:
        tc = ctx.enter_context(tile.TileContext(nc))
        res = ctx.enter_context(tc.tile_pool(name="res", bufs=1))
        pml = ctx.enter_context(tc.tile_pool(name="pml", bufs=3, space="PSUM"))
        pcol = ctx.enter_context(tc.tile_pool(name="pcol", bufs=2,
                                              space="PSUM"))

        e1t_sb = res.tile([128, IT, KK, 2, 128], f8)
        e2t_sb = res.tile([128, JC, KK, 2, 512], f8)
        exps_sb = res.tile([128, IT, N], bf16)
        colacc = res.tile([128, N], bf16)
        rows_sb = res.tile([128, IT * NGR], fp32)
        colp_sb = res.tile([1, N], fp32)
        ones_bf = res.tile([128, 1], bf16)
        scratch = res.tile([128, 512], bf16)

        # memsets first so the PE warm-up can start immediately
        nc.vector.memset(ones_bf, 1.0)
        nc.vector.memset(scratch, 1.0)

        # ---- input DMAs: one ordered stream on the sync queue ----
        # A single queue keeps chunk completion in issue order at full HBM
        # bandwidth (split queues fair-share and delay the first chunk),
        # and sync is otherwise idle so the scalar queue stays free for
        # the ACT stream.  gpsimd is avoided entirely: touching it makes
        # its 12 exit DRAINs cost ~1.2us each (DGE drain) instead of ~0.2us.
        SL1 = KK * 2 * 128                      # bytes per i-tile slice
        nc.sync.dma_start(out=e1t_sb[:, 0], in_=e1t_d[:, 0:SL1])
        nc.sync.dma_start(out=e2t_sb[:, 0], in_=e2t_d[:, 0:KK * 2 * 512])
        nc.sync.dma_start(out=e1t_sb[:, 1:], in_=e1t_d[:, SL1:])
        for jc in range(1, JC):
            nc.sync.dma_start(out=e2t_sb[:, jc],
                              in_=e2t_d[:, ts(jc, KK * 2 * 512)])

        # ---- PE warm-up: HAM clock-gate needs ~3.4us of busy to unthrottle,
        # and these also bridge the PE until the first input chunks land.
        # Short N=128 matmuls (~107ns cold) keep the bridge fine-grained so
        # the first real matmul starts almost immediately once data lands.
        warm = pcol.tile([128, 512], fp32, tag="cps")
        for w in range(NWARM):
            nc.tensor.matmul(warm[0:1, 0:128], lhsT=ones_bf,
                             rhs=scratch[:, 0:128], start=True, stop=True)

        # ---- main loop: j-group outer, i-tile inner ----
        # colsum ones-matmuls are emitted 1 group late (so the PE queue
        # never waits on the DVE adds feeding them) and the PSUM->SBUF
        # copies one slot later still (so the DVE queue never blocks on a
        # not-yet-run ones-matmul while later adds wait behind it).  Copy
        # emission precedes the matmul that reuses the rotating pcol
        # buffer, keeping Tile's WAR tracking sound.
        GROUPS = [[0], [1], [2, 3], [4, 5], [6], [7]]
        NGR = len(GROUPS)
        cps_tiles = {}

        def emit_colsum_mm(gi):
            for c in GROUPS[gi]:
                cps = pcol.tile([128, 512], fp32, tag="cps")
                cps_tiles[c] = cps
                nc.tensor.matmul(cps[0:1, :], lhsT=ones_bf,
                                 rhs=colacc[:, ts(c, 512)],
                                 start=True, stop=True)

        def emit_colsum_copy(gi, engine=None):
            for c in GROUPS[gi]:
                eng = engine if engine is not None else nc.vector
                if engine is None:
                    eng.tensor_copy(out=colp_sb[:, ts(c, 512)],
                                    in_=cps_tiles.pop(c)[0:1, :])
                else:
                    eng.copy(out=colp_sb[:, ts(c, 512)],
                             in_=cps_tiles.pop(c)[0:1, :])
                # piecewise output DMA: only the last chunk's 2KB transfer
                # remains on the critical tail
                nc.sync.dma_start(out=colp_d[:, ts(c, 512)],
                                  in_=colp_sb[:, ts(c, 512)])

        for gi, chunks in enumerate(GROUPS):
            nch = len(chunks)
            for it in range(IT):
                pg = pml.tile([128, 2, 512], fp32, tag="pg")
                for kk in range(KK):
                    for j2, jc in enumerate(chunks):
                        nc.tensor.matmul(
                            pg[:, j2, :],
                            lhsT=e1t_sb[:, it, kk, :, :],
                            rhs=e2t_sb[:, jc, kk, :, :],
                            start=(kk == 0), stop=(kk == KK - 1),
                            perf_mode=DR)
                slot = it * NGR + gi
                if nch == 2:
                    # wide ACT amortizes the ~300ns fixed cost; accum_out
                    # gives the row partial sums in fp32 for free
                    nc.scalar.activation(
                        out=exps_sb[:, it, chunks[0] * 512:
                                    (chunks[-1] + 1) * 512],
                        in_=pg, func=AF.Exp, scale=INV_T,
                        accum_out=rows_sb[:, slot:slot + 1])
                else:
                    # narrow startup group: skip the accumulator read
                    # (ACT headroom is tight) and row-reduce on the DVE
                    nc.scalar.activation(
                        out=exps_sb[:, it, ts(chunks[0], 512)],
                        in_=pg[:, 0, :], func=AF.Exp, scale=INV_T)
                    nc.vector.reduce_sum(
                        out=rows_sb[:, slot:slot + 1],
                        in_=exps_sb[:, it, ts(chunks[0], 512)],
                        axis=mybir.AxisListType.X)
                # column-sum accumulation for this group (DVE, overlapped)
                if it >= 1:
                    for jc in chunks:
                        sl = ts(jc, 512)
                        if it == 1:
                            nc.vector.tensor_add(out=colacc[:, sl],
                                                 in0=exps_sb[:, 0, sl],
                                                 in1=exps_sb[:, 1, sl])
                        else:
                            nc.vector.tensor_add(out=colacc[:, sl],
                                                 in0=colacc[:, sl],
                                                 in1=exps_sb[:, it, sl])
            if gi >= 2:
                emit_colsum_copy(gi - 2)
            if gi >= 1:
                emit_colsum_mm(gi - 1)
        nc.sync.dma_start(out=rows_d[:, :], in_=rows_sb)
        emit_colsum_copy(NGR - 2)
        emit_colsum_mm(NGR - 1)
        # last chunk's copy on the (now idle) scalar engine so it runs in
        # parallel with any remaining DVE work
        emit_colsum_copy(NGR - 1, engine=nc.scalar)
    return _legalize_waits(nc) if legalize else nc


def _get_nc():
    if "nc" not in _CACHE:
        _CACHE["nc"] = build_nc()
    return _CACHE["nc"]


def _run(in_maps, trace=False, **kw):
    from concourse.bass_utils import run_bass_kernel_spmd
    return run_bass_kernel_spmd(_get_nc(), in_maps,
                                core_ids=list(range(NCORES)),
                                trace=trace, **kw)


def kernel(embeddings1, embeddings2, _trace=False, _full_result=False):
    e1 = np.asarray(embeddings1, dtype=np.float32)
    e2 = np.asarray(embeddings2, dtype=np.float32)
    assert e1.shape == (N, D) and e2.shape == (N, D)
    f8 = ml_dtypes.float8_e4m3

    e1n = e1.astype(np.float64)
    e1n /= np.maximum(np.linalg.norm(e1n, axis=1, keepdims=True), 1e-12)
    e2n = e2.astype(np.float64)
    e2n /= np.maximum(np.linalg.norm(e2n, axis=1, keepdims=True), 1e-12)
    ldiag = INV_T * np.sum(e1n * e2n, axis=1)

    q1 = e1n.astype(np.float32).astype(f8)
    q2 = e2n.astype(np.float32).astype(f8)
    # e2t[p, jc, kk, pr, jl] = q2.T[kk*256+pr*128+p, jc*512+jl]
    e2t = np.ascontiguousarray(
        q2.T.reshape(KK, 2, 128, JC, 512)
            .transpose(2, 3, 0, 1, 4).reshape(128, -1))

    in_maps = []
    for c in range(NCORES):
        q1c = q1[c * CH:(c + 1) * CH]
        # e1t[p, it, kk, pr, il] = q1c.T[kk*256+pr*128+p, it*128+il]
        e1t = np.ascontiguousarray(
            q1c.T.reshape(KK, 2, 128, IT, 128).transpose(2, 3, 0, 1, 4)
               .reshape(128, -1))
        in_maps.append({"e1t": e1t, "e2t": e2t})

    bres = _run(in_maps, trace=_trace)
    outs = bres.results

    NGR_H = 6
    rows = np.concatenate([
        np.asarray(o["rows"], dtype=np.float64)
          .reshape(128, IT, NGR_H).sum(axis=2).T.reshape(-1)
        for o in outs])
    colsum = np.zeros(N, dtype=np.float64)
    for o in outs:
        colsum += np.asarray(o["colp"], dtype=np.float64).reshape(-1)

    ed = np.exp(ldiag)
    row_denom = rows - ed
    col_denom = colsum - ed
    sim12 = float(np.sum(ldiag - np.log(row_denom)))
    sim21 = float(np.sum(ldiag - np.log(col_denom)))
    result = (np.float32(-sim12), np.float32(-sim21))
    if _full_result:
        return result, bres
    return result
